# revision 26
# baseline (speedup 1.0000x reference)
"""VQ codebook assignment + nearest upsample on 8 NeuronCores.

Problem (per domain): given features f [B=4, C=256, H=64, W=128] and
centroids c [K=19, C=256], compute argmin_k ||f[b,:,h,w] - c_k||^2 and
nearest-upsample the [64,128] index map to [512,1024] (8x in each axis).
Two independent domains (cross-assigned centroids) x 4 batches = 8 cores,
one batch-image per core, no cross-core communication.

Precision scheme (measured on the actual inputs): features are quantized
to fp16 on the host (halves HBM traffic and PE passes); centroids are
carried as an fp16 hi+lo pair (hi + lo reconstructs fp32-fidelity
weights), and the bias -||c||^2/2 is computed on the host in fp64 from
the reconstructed centroids and added exactly on the DVE in fp32.
Scores stay fp32 throughout.  Measured rel_err 1.33e-2 (< 2e-2 gate,
identical flip count to the numpy prediction); bf16 or fp16-centroid
variants measure over the gate and are not used.

Per-core pipeline, per supergroup of 4 x 512-pixel chunks:
  1. 16 fp16 matmuls accumulate cross[k,px] into one PSUM bank: the w
     hi/lo blocks (padded to 32 cols) are stationary; each of the 4
     chunks streams through its own 32-col array strip (tile_position
     col tiling) so 4 matmuls run concurrently.
  2. One ScalarE copy moves the [128,512] scores PSUM->SBUF (plain Copy
     is bit-exact; the Identity-LUT bias path is not).
  3. 4 PE transposes flip [128,128] score slices into pixel-partition
     layout [128px, strips x 32] in PSUM.
  4. DVE argmax on strided views that skip the 13 pad columns per
     strip: bias-add (exact fp32 TT), reduce_max, is_ge,
     (eq*-1024 + (iota+1024)), reduce_min -> the index itself --
     exact first-match tie semantics in fp32.  Supergroups 0+1 are
     argmax'd as one fused pair (halved per-op overhead); supergroups
     2+3 run singly so the pipeline drain after the last DMA is short.
Tail per 32-row half: permuted copy fixes the (c,j) supergroup ordering
back to linear h, DVE 32x32 transpose + block copies build the [h,w]
int8 map, ScalarE broadcast-copies the 8x x-replication, and a single
stride-0-source HWDGE store per half performs the 8x y-replication.
The mask travels as int8 (indices 0..18, lossless) and the host widens
to int32.

The queues matter: all feature loads ride the sync HWDGE queue; the
scalar queue carries only tiny setup DMAs and the ScalarE copies, so
per-supergroup pipelines are not serialized behind load triggers
(sequencers are FIFO).  A burst of dummy matmuls on scratch warms the
PE HAM clock gate through the DMA-in phase when the part cooperates;
the schedule does not depend on it.
"""

import numpy as np

import concourse.bass as bass
import concourse.mybir as mybir
import concourse.tile as tile
from concourse import bacc
from concourse.bass import ds
from concourse.bass_utils import run_bass_kernel_spmd

F32 = mybir.dt.float32
F16 = mybir.dt.float16
I32 = mybir.dt.int32
I8 = mybir.dt.int8

B = 4
C = 256
H, W = 64, 128
K = 19
KP = 32               # w block padded to 32 columns (one array strip)
HL, WL = 512, 1024
NPIX = H * W          # 8192
CH = 512              # matmul moving chunk (pixels)
SG = 4 * CH           # supergroup: 4 chunks processed concurrently
NSG = NPIX // SG      # 4 supergroups
UP = HL // H          # 8x upsample
BIG = 1024.0
FWC = 2 * KP + NPIX   # fw columns: [w_hi32 | w_lo32 | pixels]

_NC_CACHE = None


def _build_nc():
    nc = bacc.Bacc("TRN2", target_bir_lowering=False, debug=False)

    fw_in = nc.dram_tensor("fw", [C, FWC], F16, kind="ExternalInput")
    bi_in = nc.dram_tensor("bi", [128, 2 * K], F32, kind="ExternalInput")
    aux2_in = nc.dram_tensor("aux2", [1, CH + 256], F16, kind="ExternalInput")
    ident_in = nc.dram_tensor("ident", [128, 128], F32, kind="ExternalInput")
    mask_out = nc.dram_tensor("mask", [HL, WL], I8, kind="ExternalOutput")

    fwv = fw_in.ap().rearrange("(a p) n -> a p n", a=2)       # [2, 128, FWC]
    outv = mask_out.ap().rearrange("(h y) x -> h y x", y=UP)  # [64, 8, 1024]

    with tile.TileContext(nc) as tc:
        with (
            tc.tile_pool(name="persist", bufs=1) as pp,
            tc.tile_pool(name="work", bufs=2) as wp,
            tc.tile_pool(name="psA", bufs=2, space="PSUM") as psA,
            tc.tile_pool(name="psP", bufs=1, space="PSUM") as psP,
            tc.tile_pool(name="psQ", bufs=2, space="PSUM") as psQ,
            tc.tile_pool(name="psS", bufs=1, space="PSUM") as psS,
        ):
            fw0 = pp.tile([128, FWC], F16, tag="fw0")
            fw1 = pp.tile([128, FWC], F16, tag="fw1")
            ident = pp.tile([128, 128], F32, tag="ident")
            aux2 = pp.tile([1, CH + 256], F16, tag="aux2")
            iotaf = pp.tile([128, K], F32, tag="iotaf")     # k + 1024
            idxv = pp.tile([128, H], F32, tag="idxv")       # [w, (sg,c,j)]
            tph = pp.tile([128, H], F32, tag="tph")         # [w, h] linear
            tmp = pp.tile([128, H], F32, tag="tmp")         # block-transposed
            idxT = pp.tile([H, W], I8, tag="idxT")          # [h, w]
            rep = pp.tile([H, WL], I8, tag="rep")

            # --- PE warm-up: dummy matmuls on zeroed scratch keep the
            # HAM activity monitor busy through the DMA-in phase so the
            # real matmul stream runs warm when the part cooperates ---
            scr = pp.tile([128, CH], F16, tag="scr")
            nc.scalar.memzero(scr)
            pssc = psS.tile([128, CH], F32, tag="pssc")
            for _ in range(18):
                nc.tensor.matmul(
                    pssc[ds(0, 32), :], scr[:, 0:32], scr,
                    start=True, stop=True, skip_group_check=True,
                )

            # --- feature loads, all on the sync HWDGE queue (the scalar
            # queue must stay clear for the per-supergroup ScalarE
            # copies); setup tensors load via the scalar queue.  The
            # last supergroups load in finer slices so their matmuls
            # overlap the final stretch of the load. ---
            nc.scalar.dma_start(ident, ident_in[:, :])
            nc.scalar.dma_start(aux2, aux2_in.ap()[:, :])
            nc.scalar.dma_start(iotaf, bi_in.ap()[:, K:2 * K])
            ld_slices = [
                ds(0, 2 * KP + CH),                    # w blocks + chunk 0
                ds(2 * KP + CH, CH),                   # sg0 chunk 1
                ds(2 * KP + 2 * CH, 2 * CH),           # sg0 chunks 2-3
                ds(2 * KP + SG, SG),                   # sg1
                ds(2 * KP + 2 * SG, SG),               # sg2
                ds(2 * KP + 3 * SG, SG),               # sg3
            ]
            for sl in ld_slices:
                nc.sync.dma_start(fw0[:, sl], fwv[0, :, sl])
                nc.sync.dma_start(fw1[:, sl], fwv[1, :, sl])

            iota_b32 = iotaf.rearrange(
                "p (g k) -> p g k", g=1, k=K
            ).to_broadcast([128, 32, K])
            iota_b16 = iotaf.rearrange(
                "p (g k) -> p g k", g=1, k=K
            ).to_broadcast([128, 16, K])
            def argmax(psTv, ncols, iota_b, outsl):
                """DVE argmax chain over [128, ncols, K] strided views
                (scores arrive pre-biased via the PE prefill)."""
                maxv = wp.tile([128, ncols], F32, tag=f"maxv{ncols}")
                nc.vector.tensor_reduce(
                    maxv, psTv, axis=mybir.AxisListType.X,
                    op=mybir.AluOpType.max,
                )
                eq = wp.tile([128, ncols, K], F32, tag=f"eq{ncols}")
                maxv_b = maxv.rearrange(
                    "p (g o) -> p g o", o=1
                ).to_broadcast([128, ncols, K])
                nc.vector.tensor_tensor(
                    eq, psTv, maxv_b, op=mybir.AluOpType.is_ge
                )
                cand = wp.tile([128, ncols, K], F32, tag=f"cand{ncols}")
                nc.vector.scalar_tensor_tensor(
                    cand, eq, -BIG, iota_b,
                    op0=mybir.AluOpType.mult, op1=mybir.AluOpType.add,
                )
                nc.vector.tensor_reduce(
                    idxv[:, outsl], cand,
                    axis=mybir.AxisListType.X, op=mybir.AluOpType.min,
                )

            def tail_half(hh):
                """Emit output rows [32*hh, 32*hh+32): permute to linear
                h, transpose to [h,w] int8, replicate 8x in x, one
                stride-0 store replicating 8x in y."""
                hsl = ds(hh * 32, 32)
                psl = ds(hh * 32, 32)
                for s in range(2):
                    sgi = 2 * hh + s
                    srcv = idxv[:, ds(sgi * 16, 16)].rearrange(
                        "p (c j) -> p c j", c=4
                    ).transpose([0, 2, 1])
                    nc.vector.tensor_copy(
                        tph[:, ds(sgi * 16, 16)].rearrange(
                            "p (j c) -> p j c", j=4
                        ),
                        srcv,
                    )
                nc.vector.transpose(tmp[:, hsl], tph[:, hsl])
                for i in range(W // 32):
                    nc.vector.tensor_copy(
                        idxT[psl, ds(32 * i, 32)],
                        tmp[ds(32 * i, 32), hsl],
                    )
                idxT_b = idxT[psl].rearrange(
                    "p (w o) -> p w o", o=1
                ).to_broadcast([32, W, UP])
                nc.scalar.copy(
                    rep[psl].rearrange("p (w x) -> p w x", w=W), idxT_b
                )
                rep_b = rep[psl].rearrange(
                    "p (o x) -> p o x", o=1
                ).to_broadcast([32, UP, WL])
                nc.sync.dma_start(outv[psl], rep_b)

            psT2 = None
            for sg in range(NSG):
                ps = psA.tile([128, CH], F32, tag="ps")
                # exact bias prefill as an fp16 hi+lo pair: cheap on PE
                # (2 x N=512 fp16 passes) and, having no data deps, it
                # runs during the DMA-in phase
                nc.tensor.matmul(
                    ps, aux2[:, ds(CH, 128)], aux2[:, ds(0, CH)],
                    start=True, stop=False,
                )
                nc.tensor.matmul(
                    ps, aux2[:, ds(CH + 128, 128)], aux2[:, ds(0, CH)],
                    start=False, stop=False,
                )
                for hf in range(2):
                    fwh = fw0 if hf == 0 else fw1
                    for part in range(2):
                        wsl = ds(part * KP, KP)
                        last = hf == 1 and part == 1
                        for j in range(4):
                            colsl = ds(2 * KP + sg * SG + j * CH, CH)
                            nc.tensor.matmul(
                                ps[ds(32 * j, 32), :],
                                fwh[:, wsl], fwh[:, colsl],
                                start=False, stop=last,
                                tile_position=(0, 32 * j),
                            )
                # plain ScalarE Copy is bit-exact
                S4 = wp.tile([128, CH], F32, tag="S4")
                nc.scalar.copy(S4, ps)
                if sg < 2:
                    # supergroups 0+1 share one psum tile; fused argmax
                    if sg == 0:
                        psT2 = psP.tile([128, 2, 4, 128], F32, tag="psT2")
                    for cc in range(4):
                        nc.tensor.transpose(
                            psT2[:, sg, cc], S4[:, ds(cc * 128, 128)], ident
                        )
                    if sg == 1:
                        psTv = psT2.rearrange(
                            "p s a b -> p (s a b)"
                        ).rearrange("p (g k) -> p g k", g=32)[:, :, 0:K]
                        argmax(psTv, 32, iota_b32, ds(0, 32))
                        tail_half(0)
                else:
                    psT = psQ.tile([128, 4, 128], F32, tag="psT")
                    for cc in range(4):
                        nc.tensor.transpose(
                            psT[:, cc], S4[:, ds(cc * 128, 128)], ident
                        )
                    psTv = psT.rearrange("p a b -> p (a b)").rearrange(
                        "p (g k) -> p g k", g=16
                    )[:, :, 0:K]
                    argmax(psTv, 16, iota_b16, ds(sg * 16, 16))
                    if sg == 3:
                        tail_half(1)

    nc.compile()
    return nc


_IDENT = None


def _prep_domain(feature, centroid):
    """Per-core inputs for one domain: 4 batches against one centroid set."""
    global _IDENT
    if _IDENT is None:
        _IDENT = np.ascontiguousarray(np.eye(128, dtype=np.float32))
    c = np.ascontiguousarray(centroid, dtype=np.float32)
    w = c.T.astype(np.float32)                                  # [C, K]
    w_hi = w.astype(np.float16)
    w_lo = (w.astype(np.float64) - w_hi.astype(np.float64)).astype(np.float16)
    # bias from the RECONSTRUCTED (quantized) centroids, in fp64
    chat = w_hi.astype(np.float64) + w_lo.astype(np.float64)    # [C, K]
    c2 = np.sum(chat * chat, axis=0)                            # [K]
    bi = np.zeros((128, 2 * K), dtype=np.float32)
    bi[:, K:2 * K] = (BIG + np.arange(K, dtype=np.float32))[None, :]
    bias = (-0.5 * c2).astype(np.float32)
    b_hi = bias.astype(np.float16)
    b_lo = (bias.astype(np.float64) - b_hi.astype(np.float64)).astype(
        np.float16
    )
    aux2 = np.zeros((1, CH + 256), dtype=np.float16)
    aux2[0, 0:CH] = 1.0
    aux2[0, CH:CH + 128] = -60000.0          # pad cols never win the max
    for j in range(4):
        aux2[0, CH + 32 * j:CH + 32 * j + K] = b_hi
        aux2[0, CH + 128 + 32 * j:CH + 128 + 32 * j + K] = b_lo
    wpad = np.zeros((C, 2 * KP), dtype=np.float16)
    wpad[:, 0:K] = w_hi
    wpad[:, KP:KP + K] = w_lo
    maps = []
    for b in range(B):
        f = np.asarray(feature[b], dtype=np.float32).reshape(C, NPIX)
        fw = np.ascontiguousarray(
            np.concatenate([wpad, f.astype(np.float16)], axis=1)
        )
        maps.append({"fw": fw, "bi": bi, "ident": _IDENT, "aux2": aux2})
    return maps


def kernel(
    feature_s2t, feature_target, label_s2t, label_target,
    centroid_s2t, centroid_target,
):
    global _NC_CACHE
    if _NC_CACHE is None:
        _NC_CACHE = _build_nc()
    nc = _NC_CACHE

    # cross assignment: s2t features vs target centroids, and vice versa
    in_maps = _prep_domain(feature_s2t, centroid_target) + _prep_domain(
        feature_target, centroid_s2t
    )
    res = run_bass_kernel_spmd(nc, in_maps, core_ids=list(range(8))).results
    mask_s2t = np.stack([res[i]["mask"] for i in range(B)]).astype(np.int32)
    mask_target = np.stack([res[B + i]["mask"] for i in range(B)]).astype(
        np.int32
    )
    return (mask_s2t, mask_target)


# revision 27
# speedup vs baseline: 1.1394x; 1.1394x over previous
"""VQ codebook assignment + nearest upsample on 8 NeuronCores.

Problem (per domain): given features f [B=4, C=256, H=64, W=128] and
centroids c [K=19, C=256], compute argmin_k ||f[b,:,h,w] - c_k||^2 and
nearest-upsample the [64,128] index map to [512,1024] (8x in each axis).
Two independent domains (cross-assigned centroids) x 4 batches = 8 cores,
one batch-image per core, no cross-core communication.

Precision scheme (measured on the actual inputs): features are quantized
to fp16 on the host (halves HBM traffic and PE passes); centroids are
carried as an fp16 hi+lo pair (hi + lo reconstructs fp32-fidelity
weights), and the bias -||c||^2/2 is computed on the host in fp64 from
the reconstructed centroids and added exactly on the DVE in fp32.
Scores stay fp32 throughout.  Measured rel_err 1.33e-2 (< 2e-2 gate,
identical flip count to the numpy prediction); bf16 or fp16-centroid
variants measure over the gate and are not used.

Per-core pipeline, per supergroup of 4 x 512-pixel chunks:
  1. 16 fp16 matmuls accumulate cross[k,px] into one PSUM bank: the w
     hi/lo blocks (padded to 32 cols) are stationary; each of the 4
     chunks streams through its own 32-col array strip (tile_position
     col tiling) so 4 matmuls run concurrently.
  2. One ScalarE copy moves the [128,512] scores PSUM->SBUF (plain Copy
     is bit-exact; the Identity-LUT bias path is not).
  3. 4 PE transposes flip [128,128] score slices into pixel-partition
     layout [128px, strips x 32] in PSUM.
  4. DVE argmax on strided views that skip the 13 pad columns per
     strip: bias-add (exact fp32 TT), reduce_max, is_ge,
     (eq*-1024 + (iota+1024)), reduce_min -> the index itself --
     exact first-match tie semantics in fp32.  Supergroups 0+1 are
     argmax'd as one fused pair (halved per-op overhead); supergroups
     2+3 run singly so the pipeline drain after the last DMA is short.
Tail per 32-row half: permuted copy fixes the (c,j) supergroup ordering
back to linear h, DVE 32x32 transpose + block copies build the [h,w]
int8 map, ScalarE broadcast-copies the 8x x-replication, and a single
stride-0-source HWDGE store per half performs the 8x y-replication.
The mask travels as int8 (indices 0..18, lossless) and the host widens
to int32.

The queues matter: all feature loads ride the sync HWDGE queue; the
scalar queue carries only tiny setup DMAs and the ScalarE copies, so
per-supergroup pipelines are not serialized behind load triggers
(sequencers are FIFO).  A burst of dummy matmuls on scratch warms the
PE HAM clock gate through the DMA-in phase when the part cooperates;
the schedule does not depend on it.
"""

import numpy as np

import concourse.bass as bass
import concourse.mybir as mybir
import concourse.tile as tile
from concourse import bacc
from concourse.bass import ds
from concourse.bass_utils import run_bass_kernel_spmd

F32 = mybir.dt.float32
F16 = mybir.dt.float16
I32 = mybir.dt.int32
I8 = mybir.dt.int8

B = 4
C = 256
H, W = 64, 128
K = 19
KP = 32               # w block padded to 32 columns (one array strip)
HL, WL = 512, 1024
NPIX = H * W          # 8192
CH = 512              # matmul moving chunk (pixels)
SG = 4 * CH           # supergroup: 4 chunks processed concurrently
NSG = NPIX // SG      # 4 supergroups
UP = HL // H          # 8x upsample
BIG = 1024.0
FWC = 2 * KP + NPIX   # fw columns: [w_hi32 | w_lo32 | pixels]

_NC_CACHE = None


def _build_nc():
    nc = bacc.Bacc("TRN2", target_bir_lowering=False, debug=False)

    fw_in = nc.dram_tensor("fw", [C, FWC], F16, kind="ExternalInput")
    bi_in = nc.dram_tensor("bi", [128, 2 * K], F32, kind="ExternalInput")
    aux2_in = nc.dram_tensor("aux2", [1, CH + 256], F16, kind="ExternalInput")
    ident_in = nc.dram_tensor("ident", [128, 128], F32, kind="ExternalInput")
    mask_out = nc.dram_tensor("mask", [HL, WL], I8, kind="ExternalOutput")

    fwv = fw_in.ap().rearrange("(a p) n -> a p n", a=2)       # [2, 128, FWC]
    outv = mask_out.ap().rearrange("(h y) x -> h y x", y=UP)  # [64, 8, 1024]

    with tile.TileContext(nc) as tc:
        with (
            tc.tile_pool(name="persist", bufs=1) as pp,
            tc.tile_pool(name="work", bufs=2) as wp,
            tc.tile_pool(name="psA", bufs=2, space="PSUM") as psA,
            tc.tile_pool(name="psP", bufs=1, space="PSUM") as psP,
            tc.tile_pool(name="psQ", bufs=2, space="PSUM") as psQ,
            tc.tile_pool(name="psS", bufs=1, space="PSUM") as psS,
        ):
            fw0 = pp.tile([128, FWC], F16, tag="fw0")
            fw1 = pp.tile([128, FWC], F16, tag="fw1")
            ident = pp.tile([128, 128], F32, tag="ident")
            aux2 = pp.tile([1, CH + 256], F16, tag="aux2")
            iotaf = pp.tile([128, K], F32, tag="iotaf")     # k + 1024
            iotaf16 = pp.tile([128, K], F16, tag="iotaf16")
            idxv = pp.tile([128, H], F32, tag="idxv")       # [w, (sg,c,j)]
            tph = pp.tile([128, H], F32, tag="tph")         # [w, h] linear
            tmp = pp.tile([128, H], F32, tag="tmp")         # block-transposed
            idxT = pp.tile([H, W], I8, tag="idxT")          # [h, w]
            rep = pp.tile([H, WL], I8, tag="rep")

            # --- PE warm-up: dummy matmuls on zeroed scratch keep the
            # HAM activity monitor busy through the DMA-in phase so the
            # real matmul stream runs warm when the part cooperates ---
            scr = pp.tile([128, CH], F16, tag="scr")
            nc.scalar.memzero(scr)
            pssc = psS.tile([128, CH], F32, tag="pssc")
            for _ in range(18):
                nc.tensor.matmul(
                    pssc[ds(0, 32), :], scr[:, 0:32], scr,
                    start=True, stop=True, skip_group_check=True,
                )

            # --- feature loads, all on the sync HWDGE queue (the scalar
            # queue must stay clear for the per-supergroup ScalarE
            # copies); setup tensors load via the scalar queue.  The
            # last supergroups load in finer slices so their matmuls
            # overlap the final stretch of the load. ---
            nc.scalar.dma_start(ident, ident_in[:, :])
            nc.scalar.dma_start(aux2, aux2_in.ap()[:, :])
            nc.scalar.dma_start(iotaf, bi_in.ap()[:, K:2 * K])
            nc.vector.tensor_copy(iotaf16, iotaf)
            ld_slices = [
                ds(0, 2 * KP + CH),                    # w blocks + chunk 0
                ds(2 * KP + CH, CH),                   # sg0 chunk 1
                ds(2 * KP + 2 * CH, 2 * CH),           # sg0 chunks 2-3
                ds(2 * KP + SG, SG),                   # sg1
                ds(2 * KP + 2 * SG, SG),               # sg2
                ds(2 * KP + 3 * SG, SG),               # sg3
            ]
            for sl in ld_slices:
                nc.sync.dma_start(fw0[:, sl], fwv[0, :, sl])
                nc.sync.dma_start(fw1[:, sl], fwv[1, :, sl])

            iota_b32 = iotaf16.rearrange(
                "p (g k) -> p g k", g=1, k=K
            ).to_broadcast([128, 32, K])
            iota_b16 = iotaf16.rearrange(
                "p (g k) -> p g k", g=1, k=K
            ).to_broadcast([128, 16, K])
            def argmax(psTv, ncols, iota_b, outsl):
                """DVE argmax chain over [128, ncols, K] strided views
                (scores arrive pre-biased via the PE prefill)."""
                maxv = wp.tile([128, ncols], F32, tag=f"maxv{ncols}")
                nc.vector.tensor_reduce(
                    maxv, psTv, axis=mybir.AxisListType.X,
                    op=mybir.AluOpType.max,
                )
                eq = wp.tile([128, ncols, K], F16, tag=f"eq{ncols}")
                maxv_b = maxv.rearrange(
                    "p (g o) -> p g o", o=1
                ).to_broadcast([128, ncols, K])
                nc.vector.tensor_tensor(
                    eq, psTv, maxv_b, op=mybir.AluOpType.is_ge
                )
                cand = wp.tile([128, ncols, K], F16, tag=f"cand{ncols}")
                nc.vector.scalar_tensor_tensor(
                    cand, eq, -BIG, iota_b,
                    op0=mybir.AluOpType.mult, op1=mybir.AluOpType.add,
                )
                nc.vector.tensor_reduce(
                    idxv[:, outsl], cand,
                    axis=mybir.AxisListType.X, op=mybir.AluOpType.min,
                )

            def tail_half(hh):
                """Emit output rows [32*hh, 32*hh+32): permute to linear
                h, transpose to [h,w] int8, replicate 8x in x, one
                stride-0 store replicating 8x in y."""
                hsl = ds(hh * 32, 32)
                psl = ds(hh * 32, 32)
                for s in range(2):
                    sgi = 2 * hh + s
                    srcv = idxv[:, ds(sgi * 16, 16)].rearrange(
                        "p (c j) -> p c j", c=4
                    ).transpose([0, 2, 1])
                    nc.vector.tensor_copy(
                        tph[:, ds(sgi * 16, 16)].rearrange(
                            "p (j c) -> p j c", j=4
                        ),
                        srcv,
                    )
                nc.vector.transpose(tmp[:, hsl], tph[:, hsl])
                for i in range(W // 32):
                    nc.vector.tensor_copy(
                        idxT[psl, ds(32 * i, 32)],
                        tmp[ds(32 * i, 32), hsl],
                    )
                idxT_b = idxT[psl].rearrange(
                    "p (w o) -> p w o", o=1
                ).to_broadcast([32, W, UP])
                nc.scalar.copy(
                    rep[psl].rearrange("p (w x) -> p w x", w=W), idxT_b
                )
                rep_b = rep[psl].rearrange(
                    "p (o x) -> p o x", o=1
                ).to_broadcast([32, UP, WL])
                nc.sync.dma_start(outv[psl], rep_b)

            psT2 = None
            for sg in range(NSG):
                ps = psA.tile([128, CH], F32, tag="ps")
                # exact bias prefill as an fp16 hi+lo pair: cheap on PE
                # (2 x N=512 fp16 passes) and, having no data deps, it
                # runs during the DMA-in phase
                nc.tensor.matmul(
                    ps, aux2[:, ds(CH, 128)], aux2[:, ds(0, CH)],
                    start=True, stop=False,
                )
                nc.tensor.matmul(
                    ps, aux2[:, ds(CH + 128, 128)], aux2[:, ds(0, CH)],
                    start=False, stop=False,
                )
                for hf in range(2):
                    fwh = fw0 if hf == 0 else fw1
                    for part in range(2):
                        wsl = ds(part * KP, KP)
                        last = hf == 1 and part == 1
                        for j in range(4):
                            colsl = ds(2 * KP + sg * SG + j * CH, CH)
                            nc.tensor.matmul(
                                ps[ds(32 * j, 32), :],
                                fwh[:, wsl], fwh[:, colsl],
                                start=False, stop=last,
                                tile_position=(0, 32 * j),
                            )
                # plain ScalarE Copy is bit-exact
                S4 = wp.tile([128, CH], F32, tag="S4")
                nc.scalar.copy(S4, ps)
                if sg < 2:
                    # supergroups 0+1 share one psum tile; fused argmax
                    if sg == 0:
                        psT2 = psP.tile([128, 2, 4, 128], F32, tag="psT2")
                    for cc in range(4):
                        nc.tensor.transpose(
                            psT2[:, sg, cc], S4[:, ds(cc * 128, 128)], ident
                        )
                    if sg == 1:
                        psTv = psT2.rearrange(
                            "p s a b -> p (s a b)"
                        ).rearrange("p (g k) -> p g k", g=32)[:, :, 0:K]
                        argmax(psTv, 32, iota_b32, ds(0, 32))
                        tail_half(0)
                else:
                    psT = psQ.tile([128, 4, 128], F32, tag="psT")
                    for cc in range(4):
                        nc.tensor.transpose(
                            psT[:, cc], S4[:, ds(cc * 128, 128)], ident
                        )
                    psTv = psT.rearrange("p a b -> p (a b)").rearrange(
                        "p (g k) -> p g k", g=16
                    )[:, :, 0:K]
                    argmax(psTv, 16, iota_b16, ds(sg * 16, 16))
                    if sg == 3:
                        tail_half(1)

    nc.compile()
    return nc


_IDENT = None


def _prep_domain(feature, centroid):
    """Per-core inputs for one domain: 4 batches against one centroid set."""
    global _IDENT
    if _IDENT is None:
        _IDENT = np.ascontiguousarray(np.eye(128, dtype=np.float32))
    c = np.ascontiguousarray(centroid, dtype=np.float32)
    w = c.T.astype(np.float32)                                  # [C, K]
    w_hi = w.astype(np.float16)
    w_lo = (w.astype(np.float64) - w_hi.astype(np.float64)).astype(np.float16)
    # bias from the RECONSTRUCTED (quantized) centroids, in fp64
    chat = w_hi.astype(np.float64) + w_lo.astype(np.float64)    # [C, K]
    c2 = np.sum(chat * chat, axis=0)                            # [K]
    bi = np.zeros((128, 2 * K), dtype=np.float32)
    bi[:, K:2 * K] = (BIG + np.arange(K, dtype=np.float32))[None, :]
    bias = (-0.5 * c2).astype(np.float32)
    b_hi = bias.astype(np.float16)
    b_lo = (bias.astype(np.float64) - b_hi.astype(np.float64)).astype(
        np.float16
    )
    aux2 = np.zeros((1, CH + 256), dtype=np.float16)
    aux2[0, 0:CH] = 1.0
    aux2[0, CH:CH + 128] = -60000.0          # pad cols never win the max
    for j in range(4):
        aux2[0, CH + 32 * j:CH + 32 * j + K] = b_hi
        aux2[0, CH + 128 + 32 * j:CH + 128 + 32 * j + K] = b_lo
    wpad = np.zeros((C, 2 * KP), dtype=np.float16)
    wpad[:, 0:K] = w_hi
    wpad[:, KP:KP + K] = w_lo
    maps = []
    for b in range(B):
        f = np.asarray(feature[b], dtype=np.float32).reshape(C, NPIX)
        fw = np.ascontiguousarray(
            np.concatenate([wpad, f.astype(np.float16)], axis=1)
        )
        maps.append({"fw": fw, "bi": bi, "ident": _IDENT, "aux2": aux2})
    return maps


def kernel(
    feature_s2t, feature_target, label_s2t, label_target,
    centroid_s2t, centroid_target,
):
    global _NC_CACHE
    if _NC_CACHE is None:
        _NC_CACHE = _build_nc()
    nc = _NC_CACHE

    # cross assignment: s2t features vs target centroids, and vice versa
    in_maps = _prep_domain(feature_s2t, centroid_target) + _prep_domain(
        feature_target, centroid_s2t
    )
    res = run_bass_kernel_spmd(nc, in_maps, core_ids=list(range(8))).results
    mask_s2t = np.stack([res[i]["mask"] for i in range(B)]).astype(np.int32)
    mask_target = np.stack([res[B + i]["mask"] for i in range(B)]).astype(
        np.int32
    )
    return (mask_s2t, mask_target)
